# revision 42
# baseline (speedup 1.0000x reference)
"""Trainium2 Bass kernel for ConstantModulationAttention.

Reference computation (B=2, S=2048, E=1024, H=16, D=64):
    sq  = sigmoid(scale_qkv).reshape(H, D)
    so  = sigmoid(scale_out)
    qkv = einsum('bse,eqhd->bqshd', x, W_qkv) * sq
    Q, K, V = qkv[:,0], qkv[:,1], qkv[:,2]
    scores  = einsum('bshd,bthd->bhst', Q, K) / 8
    w       = softmax(where(causal, scores, -inf), axis=-1)
    attn    = einsum('bhst,bthd->bshd', w, V)
    out     = einsum('bshd,hde->bse', attn, W_out) * so

Sharding: 8 cores = 2 (batch) x 4 (head groups of 4 heads).  Each core
computes a partial output over its 4 heads; the host sums the 4 partials
per batch (the tensor-parallel reduce) and stacks the 2 batches.  The
host hands each core x^T in bf16 (layout marshalling, part of sharding).

Per-core device program (all PE matmuls bf16 except tiny f32r helpers):
  xT    [E, S]   bf16, DMA'd straight into SBUF (no on-chip casts)
  QT,KT [HD, S]  = (W^T x^T), gated per-partition (hd) on eviction
  V     [S, HD]  natural layout, gated, augmented with a ones column per
                 head (V_aug [S, 65]) so the attention matmul's 65th row
                 yields the softmax denominator L
  scores: heads processed in PAIRS -- even head on PE rows 0-63, odd
                 head on rows 64-127 (K=64 row-tiled matmuls, concurrent
                 in the array) so the pair streams in the time of one
  exp on ACT (scale=1/8 folds the 1/sqrt(D)), bf16 out; ACT does exp ONLY
  attnT [65, q]  = V_aug^T @ exp accumulated over k; row 64 = L
  attnN [hd, s]  = attnT[0:64] * (1/L) (bf16 PE broadcast of the
                 [1,QC] approx-reciprocal, applied on DVE)
  out   [s, e]   = attnN^T @ (W_out * so), bf16 partial DMA'd out

ALL stream-side PSUM (projections, scores, outproj, broadcasts) shares
ONE 5-deep rotating pool, so there is no pool barrier between the
projection and attention phases and no PE gap at the seam (a PE gap of
>~2us re-arms the HAM throttle and costs 10-20us at half clock).
Outproj matmuls, normalizations, and the deferred chunk-3 K/V
projections are queued and popped one per k-tile slot inside the
attention loop, so the PE stays dense through the late, filler-starved
chunks as well.
"""

from collections import deque
from contextlib import ExitStack

import numpy as np

import concourse.bass as bass  # noqa: F401
import concourse.tile as tile
from concourse import bacc, mybir
from concourse.bass_utils import run_bass_kernel_spmd

F32 = mybir.dt.float32
F32R = mybir.dt.float32r
BF16 = mybir.dt.bfloat16

B, S, E, H, D = 2, 2048, 1024, 16, 64
P = 128
HC = H // 4          # 4 heads per core
HDC = HC * D         # 256 per-core head-dim features
NE = E // P          # 8 e-tiles
NS = S // P          # 16 s-tiles
QC = 512             # q-chunk width
NQ = S // QC         # 4 q-chunks
LAG = 2              # attn matmul trails the scores by LAG k-tiles


def r(ap):
    """bitcast an fp32 AP to fp32r for full-rate PE matmuls."""
    return ap.bitcast(F32R)


def build(tc, out_ap, xt_ap, wq_ap, wk_ap, wv_ap, wo_ap, sq_ap, so_ap):
    nc = tc.nc
    Exp = mybir.ActivationFunctionType.Exp
    Sigm = mybir.ActivationFunctionType.Sigmoid

    with ExitStack() as es:
        # ---------------- pools (no phase scoping) ----------------
        cpool = es.enter_context(tc.tile_pool(name="const", bufs=1))
        wopool = es.enter_context(tc.tile_pool(name="wo", bufs=1))
        qkpool = es.enter_context(tc.tile_pool(name="qk", bufs=1))
        vpool = es.enter_context(tc.tile_pool(name="vp", bufs=1))
        anpool = es.enter_context(tc.tile_pool(name="an", bufs=1))
        xtpool = es.enter_context(tc.tile_pool(name="xt", bufs=1))
        kvwpool = es.enter_context(tc.tile_pool(name="kvw", bufs=1))
        expool = es.enter_context(tc.tile_pool(name="ex", bufs=8))
        recpool = es.enter_context(tc.tile_pool(name="rec", bufs=4))
        outpool = es.enter_context(tc.tile_pool(name="outp", bufs=4))

        # first-needed transfers go ahead of everything on the Sync queue
        wq_src = wq_ap.rearrange("(u p) f -> p u f", p=P)
        wk_src = wk_ap.rearrange("(u p) f -> p u f", p=P)
        wv_src = wv_ap.rearrange("(u p) f -> p u f", p=P)
        x_src = xt_ap.rearrange("(u p) s -> p u s", p=P)

        wq_B = kvwpool.tile([P, NE, HDC], BF16, tag="wqB", name="wqB")
        wk_B = kvwpool.tile([P, NE, HDC], BF16, tag="wkB", name="wkB")
        wv_B = kvwpool.tile([P, NE, HDC], BF16, tag="wvB", name="wvB")
        xB = xtpool.tile([P, NE, S], BF16, tag="xB", name="xB")
        wq_b = [wq_B[:, u, :] for u in range(NE)]
        wk_b = [wk_B[:, u, :] for u in range(NE)]
        wv_b = [wv_B[:, u, :] for u in range(NE)]
        xb = [xB[:, u, :] for u in range(NE)]

        # gates first (3 tiny issues; the sigmoids must be ready before
        # the first QK eviction or the PSUM bank rotation stalls the PE)
        sgcol_raw = cpool.tile([P, 2], F32, tag="sgcol_raw", name="sgcol_raw")
        sgcol = cpool.tile([P, 2], F32, tag="sgcol", name="sgcol")
        nc.sync.dma_start(out=sgcol_raw[:, :],
                          in_=sq_ap.rearrange("(v p) -> p v", p=P))
        nc.scalar.activation(sgcol[:, :], sgcol_raw[:, :], Sigm)
        sgrow_raw = cpool.tile([1, HDC], F32, tag="sgrow_raw", name="sgrow_raw")
        sgrow = cpool.tile([1, HDC], F32, tag="sgrow", name="sgrow")
        nc.sync.dma_start(out=sgrow_raw[:, :], in_=sq_ap[None, :])
        nc.scalar.activation(r(sgrow[:, :]), sgrow_raw[:, :], Sigm)
        sorow_raw = cpool.tile([1, E], F32, tag="sorow_raw", name="sorow_raw")
        sorow = cpool.tile([1, E], F32, tag="sorow", name="sorow")
        nc.sync.dma_start(out=sorow_raw[:, :], in_=so_ap[None, :])
        nc.scalar.activation(r(sorow[:, :]), sorow_raw[:, :], Sigm)

        # first-chunk data, then the K/V weights (needed by ~15us),
        # then the remaining x chunks and W_out
        for half in (slice(0, 4), slice(4, 8)):
            nc.sync.dma_start(out=wq_B[:, half, :], in_=wq_src[:, half, :])
            nc.sync.dma_start(out=xB[:, half, 0:QC], in_=x_src[:, half, 0:QC])
        nc.sync.dma_start(out=wk_B[:, :, :], in_=wk_src[:, :, :])
        nc.sync.dma_start(out=wv_B[:, :, :], in_=wv_src[:, :, :])
        nc.sync.dma_start(out=xB[:, :, QC : 2 * QC],
                          in_=x_src[:, :, QC : 2 * QC])
        nc.sync.dma_start(out=xB[:, :, 2 * QC : 3 * QC],
                          in_=x_src[:, :, 2 * QC : 3 * QC])
        nc.sync.dma_start(out=xB[:, :, 3 * QC : 4 * QC],
                          in_=x_src[:, :, 3 * QC : 4 * QC])

        ones_row = cpool.tile([1, P], F32, tag="ones_row", name="ones_row")
        ones_raw = cpool.tile([1, P], F32, tag="ones_raw", name="ones_raw")
        nc.vector.memset(ones_raw[:, :], 1.0)
        nc.vector.tensor_copy(r(ones_row[:, :]), ones_raw[:, :])
        ones_b = cpool.tile([1, P], BF16, tag="ones_b", name="ones_b")
        nc.vector.tensor_copy(ones_b[:, :], ones_raw[:, :])
        ones4 = cpool.tile([P, HC], F32, tag="ones4", name="ones4")
        nc.vector.memset(ones4[:, :], 1.0)

        sqv_b = cpool.tile([P, HDC], F32, tag="sqv_b", name="sqv_b")
        so_bb = cpool.tile([P, E], BF16, tag="so_bb", name="so_bb")

        wo_b = [wopool.tile([P, E], BF16, tag=f"wob{c}", name=f"wob{c}")
                for c in range(2)]
        for c in range(2):
            nc.sync.dma_start(out=wo_b[c][:, :],
                              in_=wo_ap[c * P : (c + 1) * P, :])

        QT = [qkpool.tile([P, S], BF16, tag=f"qt{v}", name=f"qt{v}")
              for v in range(2)]
        KT = [qkpool.tile([P, S], BF16, tag=f"kt{v}", name=f"kt{v}")
              for v in range(2)]
        Vt = [vpool.tile([P, HC * 65], BF16, tag=f"v{t}", name=f"v{t}")
              for t in range(NS)]
        attnN = [anpool.tile([P, S], BF16, tag=f"an{c}", name=f"an{c}")
                 for c in range(2)]

        # ones columns of V_aug (DVE-only, no DMA dependency)
        for t in range(NS):
            nc.vector.tensor_copy(
                Vt[t][:, :].rearrange("p (h c) -> p h c", c=65)[:, :, 64],
                ones4[:, :])

        # ---------------- projection helpers ----------------
        def qk_proj(pool, tag, wname, wt, out_t, v, c, lo=0, hi=NE, ps=None):
            if ps is None:
                ps = pool.tile([P, QC], F32, tag=tag, name=f"ps{wname}{v}_{c}")
            for u in range(lo, hi):
                nc.tensor.matmul(
                    ps[:, :],
                    wt[u][:, v * P : (v + 1) * P],
                    xb[u][:, c * QC : (c + 1) * QC],
                    start=(u == 0), stop=(u == NE - 1),
                    skip_group_check=True)
            if hi == NE:
                nc.vector.tensor_scalar_mul(
                    out_t[v][:, c * QC : (c + 1) * QC], ps[:, :],
                    sgcol[:, v : v + 1])
            return ps

        def v_proj(pool, tag, t, lo=0, hi=NE, ps=None):
            if ps is None:
                ps = pool.tile([P, QC], F32, tag=tag, name=f"psv{t}")
            for u in range(lo, hi):
                nc.tensor.matmul(
                    ps[:, 0:HDC],
                    xb[u][:, t * P : (t + 1) * P],
                    wv_b[u][:, :],
                    start=(u == 0), stop=(u == NE - 1),
                    skip_group_check=True)
            if hi == NE:
                nc.vector.tensor_mul(
                    Vt[t][:, :].rearrange("p (h c) -> p h c", c=65)[:, :, 0:64],
                    ps[:, 0:HDC].rearrange("p (h d) -> p h d", d=D),
                    sqv_b[:, :].rearrange("p (h d) -> p h d", d=D))
            return ps

        # ---------------- phase 1: QKV projections ----------------
        # chunk 3's K and V projections are deferred into the attention
        # loop (they are only consumed by attention chunk 3, and they
        # give the late, filler-starved slots real K=128 PE work).
        # ALL stream-side PSUM (projections, scores, outproj, broadcasts,
        # deferred projections) shares ONE 4-deep rotation on banks 0-3;
        # the attention accumulators get fresh banks 4-7 that phase 1
        # never touches.  The last projection tiles are >=4 allocations
        # before the first scores tile, so attention starts with no WAR
        # wait on phase 1 (a PE gap at the seam re-arms the HAM throttle
        # and costs 10-20us at half clock).
        scps = es.enter_context(tc.tile_pool(name="ps_sc", bufs=4,
                                             space="PSUM"))
        atps = es.enter_context(tc.tile_pool(name="ps_at", bufs=4,
                                             space="PSUM"))

        for c in range(NQ):
            for wname, wt, out_t in (("q", wq_b, QT), ("k", wk_b, KT)):
                if c >= 2 and wname == "k":
                    continue  # deferred into phase 2
                for v in range(2):
                    qk_proj(scps, "sc", wname, wt, out_t, v, c)

            if c == 0:
                # gate broadcasts over partitions via PE (K=1 matmuls);
                # placed after chunk 0's projections so the PE doesn't
                # head-of-line block on the gate DMAs
                bq = scps.tile([P, QC], F32, tag="sc", name="bq")
                nc.tensor.matmul(bq[:, 0:HDC], r(ones_row[:, :]),
                                 r(sgrow[:, :]), start=True, stop=True)
                nc.vector.tensor_copy(sqv_b[:, :], bq[:, 0:HDC])
                for cc in range(2):
                    bo = scps.tile([P, QC], F32, tag="sc", name=f"bo{cc}")
                    nc.tensor.matmul(bo[:, :], r(ones_row[:, :]),
                                     r(sorow[:, cc * QC : (cc + 1) * QC]),
                                     start=True, stop=True)
                    nc.vector.tensor_copy(
                        so_bb[:, cc * QC : (cc + 1) * QC], bo[:, :])
                for cc in range(2):
                    nc.vector.tensor_mul(wo_b[cc][:, :], wo_b[cc][:, :],
                                         so_bb[:, :])

            for t in range(4 * c, 4 * c + 4):
                if c >= 2:
                    continue  # deferred into phase 2
                v_proj(scps, "sc", t)

        # ---------------- phase 2: attention + output projection ----------------
        # queued PE work (outproj matmuls, normalizations, deferred
        # chunk-3 projections), popped one item per k-tile slot so the
        # PE never runs dry mid-attention
        # norms go through a priority queue: each must pop within ~2
        # blocks of creation or the attn-accumulator bank rotation wraps
        # onto an un-normalized accumulator and the PE FIFO deadlocks
        # (the norm's broadcast matmul would sit behind the very matmul
        # waiting for it)
        prio = deque()
        fillers = deque()

        def pop_filler(reserve=0):
            if prio:
                prio.popleft()()
            elif len(fillers) > reserve:
                fillers.popleft()()

        def emit_outproj(i):
            for t in range(4 * i, 4 * i + 4):
                ot = outpool.tile([P, E], BF16, tag="ot", name=f"ot{t}")
                for eh in range(2):
                    def op_item(t=t, eh=eh, ot=ot):
                        po = scps.tile([P, QC], F32, tag="sc",
                                       name=f"po{t}_{eh}")
                        for c in range(2):
                            nc.tensor.matmul(
                                po[:, :],
                                attnN[c][:, t * P : (t + 1) * P],
                                wo_b[c][:, eh * QC : (eh + 1) * QC],
                                start=(c == 0), stop=(c == 1),
                                skip_group_check=True)
                        nc.vector.tensor_copy(
                            ot[:, eh * QC : (eh + 1) * QC], po[:, :])
                        if eh == 1:
                            nc.sync.dma_start(
                                out=out_ap[t * P : (t + 1) * P, :],
                                in_=ot[:, :])
                    fillers.append(op_item)

        def defer_k(c):
            for v in range(2):
                state = {}

                def ka(v=v, c=c, state=state):
                    state["ps"] = qk_proj(scps, "sc", "k", wk_b, KT, v, c,
                                          0, 4)

                def kb(v=v, c=c, state=state):
                    qk_proj(scps, "sc", "k", wk_b, KT, v, c, 4, NE,
                            state["ps"])
                fillers.append(ka)
                fillers.append(kb)

        def defer_v(ts):
            for t in ts:
                state = {}

                def va(t=t, state=state):
                    state["ps"] = v_proj(scps, "sc", t, 0, 4)

                def vb(t=t, state=state):
                    v_proj(scps, "sc", t, 4, NE, state["ps"])
                fillers.append(va)
                fillers.append(vb)

        def emit_deferred_proj():
            # chunks 2+3's K and V projections, split into half-items so
            # a single pop doesn't starve the exp pipeline; ordered by
            # first-use time (V-c2 at attn (2,0) j=8, K-c2 at scores
            # (2,0) j=8, the rest in chunk 3)
            defer_v(range(8, 12))
            defer_k(2)
            defer_k(3)
            defer_v(range(12, 16))

        # the deferred chunk-3 projections are ready to run from the very
        # first attention slot (their inputs landed with the phase-1
        # DMAs) -- emitting them here keeps the otherwise filler-less
        # chunks 0-1 dense so the HAM clock gate never re-arms early
        emit_deferred_proj()

        for i in range(NQ):
            for p in range(2):
                # head pair (2p, 2p+1): even head on PE rows 0-63, odd
                # head on rows 64-127 -- the two K=64 score matmuls
                # row-tile into disjoint quadrant rows and run
                # concurrently in the array
                kt, qt = KT[p], QT[p]
                njs = 4 * i + 4
                accA = atps.tile([65, QC], F32, tag="at", name=f"aA{i}{p}")
                accB = atps.tile([65, QC], F32, tag="at", name=f"aB{i}{p}")
                hA, hB = 2 * p, 2 * p + 1
                exs = {}

                def attn_mm(j, accA=accA, accB=accB, njs=njs, exs=exs,
                            hA=hA, hB=hB):
                    exA, exB, off, wdt = exs.pop(j)
                    nc.tensor.matmul(
                        accA[:, off : off + wdt],
                        Vt[j][:, hA * 65 : hA * 65 + 65],
                        exA[:, 0:wdt], start=(j == 0),
                        stop=(j == njs - 1), skip_group_check=True)
                    nc.tensor.matmul(
                        accB[:, off : off + wdt],
                        Vt[j][:, hB * 65 : hB * 65 + 65],
                        exB[:, 0:wdt], start=(j == 0),
                        stop=(j == njs - 1), skip_group_check=True)

                for j in range(njs):
                    diag = j >= 4 * i
                    off = max(0, j * P - i * QC)
                    wdt = QC - off
                    spA = scps.tile([P, QC], F32, tag="sc",
                                    name=f"sA{i}_{p}_{j}")
                    spB = scps.tile([P, QC], F32, tag="sc",
                                    name=f"sB{i}_{p}_{j}")
                    exA = expool.tile([P, QC], BF16, tag="ex",
                                      name=f"eA{i}_{p}_{j}")
                    exB = expool.tile([P, QC], BF16, tag="ex",
                                      name=f"eB{i}_{p}_{j}")
                    nc.tensor.matmul(
                        spA[:, 0:wdt],
                        kt[0:D, j * P : (j + 1) * P],
                        qt[0:D, i * QC + off : i * QC + off + wdt],
                        start=True, stop=True)
                    nc.tensor.matmul(
                        spB[:, 0:wdt],
                        kt[D : 2 * D, j * P : (j + 1) * P],
                        qt[D : 2 * D, i * QC + off : i * QC + off + wdt],
                        start=True, stop=True)
                    nc.scalar.activation(exA[:, 0:wdt], spA[:, 0:wdt],
                                         Exp, scale=0.125)
                    nc.scalar.activation(exB[:, 0:wdt], spB[:, 0:wdt],
                                         Exp, scale=0.125)
                    if diag:
                        # zero the q<k half of the leading [128,128]
                        # window on GPSIMD (otherwise idle)
                        for exx in (exA, exB):
                            nc.gpsimd.affine_select(
                                out=exx[:, 0:P], in_=exx[:, 0:P],
                                compare_op=mybir.AluOpType.is_ge,
                                fill=0.0, base=0,
                                pattern=[[1, P]], channel_multiplier=-1)
                    exs[j] = (exA, exB, off, wdt)
                    if j >= LAG:
                        attn_mm(j - LAG)
                    if j >= 2:
                        # one pop per slot: filler supply (72 items) is
                        # matched to the eligible slots (64), so the
                        # queue never runs dry mid-attention (an
                        # underfilled stretch re-arms the HAM throttle).
                        # In the very last block hold 4 items in reserve
                        # -- they pop right after the final attention
                        # matmuls and cover the normalization chain's
                        # DVE latency, so the PE has no gap going into
                        # the tail
                        reserve = 4 if (i == NQ - 1 and p == 1) else 0
                        pop_filler(reserve)
                        if i <= 1:
                            # chunks 0-1 have short, sparse slots (small
                            # q-windows): double the filler rate there so
                            # the PE stays dense enough to hold HAM at
                            # full clock; later chunks are naturally
                            # dense and live off the remaining supply
                            pop_filler(reserve)
                attn_mm(njs - 2)
                attn_mm(njs - 1)

                # stage the L rows down to partition 0, approx-invert the
                # [1, QC] rows, round to bf16; the deferred norm
                # broadcasts 1/L on the PE (bf16 K=1 matmul) and applies
                # it on DVE
                lrA = recpool.tile([1, QC], F32, tag="lr", name=f"lA{i}{p}")
                lrB = recpool.tile([1, QC], F32, tag="lr", name=f"lB{i}{p}")
                nc.vector.tensor_copy(lrA[:, :], accA[64:65, :])
                nc.vector.tensor_copy(lrB[:, :], accB[64:65, :])
                liA = recpool.tile([1, QC], F32, tag="li", name=f"iA{i}{p}")
                liB = recpool.tile([1, QC], F32, tag="li", name=f"iB{i}{p}")
                nc.vector.reciprocal_approx_fast(liA[:, :], lrA[:, :])
                nc.vector.reciprocal_approx_fast(liB[:, :], lrB[:, :])
                lbA = recpool.tile([1, QC], BF16, tag="lb", name=f"bA{i}{p}")
                lbB = recpool.tile([1, QC], BF16, tag="lb", name=f"bB{i}{p}")
                nc.vector.tensor_copy(lbA[:, :], liA[:, :])
                nc.vector.tensor_copy(lbB[:, :], liB[:, :])

                def norm(acc, lb, hr, p=p, i=i):
                    def run(acc=acc, lb=lb, hr=hr, p=p, i=i):
                        bc = scps.tile([P, QC], F32, tag="sc",
                                       name=f"bc{i}_{p}_{hr}")
                        nc.tensor.matmul(bc[:, :], ones_b[:, :], lb[:, :],
                                         start=True, stop=True)
                        bcs = recpool.tile([64, QC], F32, tag="bcs",
                                           name=f"bs{i}_{p}_{hr}")
                        # ACT is half-idle in attention; staging the
                        # broadcast there keeps the DVE queue short
                        nc.scalar.copy(bcs[:, :], bc[0:64, :])
                        nc.vector.tensor_mul(
                            attnN[p][hr : hr + D, i * QC : (i + 1) * QC],
                            acc[0:64, :], bcs[:, :])
                    return run

                prio.append(norm(accA, lbA, 0))
                prio.append(norm(accB, lbB, 64))

            emit_outproj(i)

        while prio or fillers:
            pop_filler()


_NC_CACHE = {}


def _get_nc():
    if "nc" in _NC_CACHE:
        return _NC_CACHE["nc"]
    nc = bacc.Bacc("TRN2", target_bir_lowering=False, debug=False,
                   enable_asserts=False, num_devices=8)
    xt_h = nc.dram_tensor("xt", [E, S], BF16, kind="ExternalInput")
    wq_h = nc.dram_tensor("wq", [E, HDC], BF16, kind="ExternalInput")
    wk_h = nc.dram_tensor("wk", [E, HDC], BF16, kind="ExternalInput")
    wv_h = nc.dram_tensor("wv", [E, HDC], BF16, kind="ExternalInput")
    wo_h = nc.dram_tensor("wo", [HDC, E], BF16, kind="ExternalInput")
    sq_h = nc.dram_tensor("sq", [HDC], F32, kind="ExternalInput")
    so_h = nc.dram_tensor("so", [E], F32, kind="ExternalInput")
    out_h = nc.dram_tensor("out", [S, E], BF16, kind="ExternalOutput")
    with tile.TileContext(nc) as tc:
        build(tc, out_h.ap(), xt_h.ap(), wq_h.ap(), wk_h.ap(), wv_h.ap(),
              wo_h.ap(), sq_h.ap(), so_h.ap())
    nc.compile()
    _NC_CACHE["nc"] = nc
    return nc


def make_in_maps(x, W_qkv, W_out, scale_qkv, scale_out, mask=None):
    import ml_dtypes
    BF = ml_dtypes.bfloat16
    in_maps = []
    sq_full = np.ascontiguousarray(scale_qkv, np.float32).reshape(H, D)
    xts = [np.ascontiguousarray(
        np.asarray(x[b], np.float32).T.astype(BF)) for b in range(B)]
    for b in range(B):
        for g in range(4):
            hs = slice(HC * g, HC * g + HC)
            in_maps.append({
                "xt": xts[b],
                "wq": np.ascontiguousarray(
                    W_qkv[:, 0, hs, :], np.float32).reshape(E, HDC).astype(BF),
                "wk": np.ascontiguousarray(
                    W_qkv[:, 1, hs, :], np.float32).reshape(E, HDC).astype(BF),
                "wv": np.ascontiguousarray(
                    W_qkv[:, 2, hs, :], np.float32).reshape(E, HDC).astype(BF),
                "wo": np.ascontiguousarray(
                    W_out[hs], np.float32).reshape(HDC, E).astype(BF),
                "sq": np.ascontiguousarray(sq_full[hs], np.float32).reshape(HDC),
                "so": np.ascontiguousarray(scale_out, np.float32),
            })
    return in_maps


def kernel(x, W_qkv, W_out, scale_qkv, scale_out, mask=None, _runner_kwargs=None):
    nc = _get_nc()
    in_maps = make_in_maps(x, W_qkv, W_out, scale_qkv, scale_out)
    kw = _runner_kwargs or {}
    res = run_bass_kernel_spmd(nc, in_maps, core_ids=list(range(8)), **kw)
    if _runner_kwargs is not None:
        kernel.last_results = res
    outs = [np.asarray(res.results[i]["out"], np.float32) for i in range(8)]
    full = np.empty((B, S, E), np.float32)
    for b in range(B):
        full[b] = outs[4 * b] + outs[4 * b + 1] + outs[4 * b + 2] + outs[4 * b + 3]
    return full


if __name__ == "__main__":
    rng = np.random.default_rng(0)
    inputs = {
        "x": rng.standard_normal((B, S, E)).astype(np.float32),
        "W_qkv": (rng.standard_normal((E, 3, H, D)).astype(np.float32) * E ** -0.5),
        "W_out": (rng.standard_normal((H, D, E)).astype(np.float32)
                  * (H * D) ** -0.5),
        "scale_qkv": (rng.standard_normal(E).astype(np.float32) * 0.02 + 1.0),
        "scale_out": (rng.standard_normal(E).astype(np.float32) * 0.02 + 1.0),
        "mask": np.tril(np.ones((S, S), bool)),
    }
    out = kernel(**inputs)
    print("kernel ran, out shape", out.shape, out.dtype)


# revision 53
# speedup vs baseline: 1.0077x; 1.0077x over previous
"""Trainium2 Bass kernel for ConstantModulationAttention.

Reference computation (B=2, S=2048, E=1024, H=16, D=64):
    sq  = sigmoid(scale_qkv).reshape(H, D)
    so  = sigmoid(scale_out)
    qkv = einsum('bse,eqhd->bqshd', x, W_qkv) * sq
    Q, K, V = qkv[:,0], qkv[:,1], qkv[:,2]
    scores  = einsum('bshd,bthd->bhst', Q, K) / 8
    w       = softmax(where(causal, scores, -inf), axis=-1)
    attn    = einsum('bhst,bthd->bshd', w, V)
    out     = einsum('bshd,hde->bse', attn, W_out) * so

Sharding: 8 cores = 2 (batch) x 4 (head groups of 4 heads).  Each core
computes a partial output over its 4 heads; the host sums the 4 partials
per batch (the tensor-parallel reduce) and stacks the 2 batches.  The
host hands each core x^T in bf16 (layout marshalling, part of sharding).

Per-core device program (all PE matmuls bf16 except tiny f32r helpers):
  xT    [E, S]   bf16, DMA'd straight into SBUF (no on-chip casts)
  QT,KT [HD, S]  = (W^T x^T), gated per-partition (hd) on eviction
  V     [S, HD]  natural layout, gated, augmented with a ones column per
                 head (V_aug [S, 65]) so the attention matmul's 65th row
                 yields the softmax denominator L
  scores: heads processed in PAIRS -- even head on PE rows 0-63, odd
                 head on rows 64-127 (K=64 row-tiled matmuls, concurrent
                 in the array) so the pair streams in the time of one
  exp on ACT (scale=1/8 folds the 1/sqrt(D)), bf16 out; ACT does exp ONLY
  attnT [65, q]  = V_aug^T @ exp accumulated over k; row 64 = L
  attnN [hd, s]  = attnT[0:64] * (1/L) (bf16 PE broadcast of the
                 [1,QC] approx-reciprocal, applied on DVE)
  out   [s, e]   = attnN^T @ (W_out * so), bf16 partial DMA'd out

ALL stream-side PSUM (projections, scores, outproj, broadcasts) shares
ONE 5-deep rotating pool, so there is no pool barrier between the
projection and attention phases and no PE gap at the seam (a PE gap of
>~2us re-arms the HAM throttle and costs 10-20us at half clock).
Outproj matmuls, normalizations, and the deferred chunk-3 K/V
projections are queued and popped one per k-tile slot inside the
attention loop, so the PE stays dense through the late, filler-starved
chunks as well.
"""

from collections import deque
from contextlib import ExitStack

import numpy as np

import concourse.bass as bass  # noqa: F401
import concourse.tile as tile
from concourse import bacc, mybir
from concourse.bass_utils import run_bass_kernel_spmd

F32 = mybir.dt.float32
F32R = mybir.dt.float32r
BF16 = mybir.dt.bfloat16

B, S, E, H, D = 2, 2048, 1024, 16, 64
P = 128
HC = H // 4          # 4 heads per core
HDC = HC * D         # 256 per-core head-dim features
NE = E // P          # 8 e-tiles
NS = S // P          # 16 s-tiles
QC = 512             # q-chunk width
NQ = S // QC         # 4 q-chunks
LAG = 2              # attn matmul trails the scores by LAG k-tiles


def r(ap):
    """bitcast an fp32 AP to fp32r for full-rate PE matmuls."""
    return ap.bitcast(F32R)


def build(tc, out_ap, xt_ap, wq_ap, wk_ap, wv_ap, wo_ap, sq_ap, so_ap):
    nc = tc.nc
    Exp = mybir.ActivationFunctionType.Exp
    Sigm = mybir.ActivationFunctionType.Sigmoid

    with ExitStack() as es:
        # ---------------- pools (no phase scoping) ----------------
        cpool = es.enter_context(tc.tile_pool(name="const", bufs=1))
        wopool = es.enter_context(tc.tile_pool(name="wo", bufs=1))
        qkpool = es.enter_context(tc.tile_pool(name="qk", bufs=1))
        vpool = es.enter_context(tc.tile_pool(name="vp", bufs=1))
        anpool = es.enter_context(tc.tile_pool(name="an", bufs=1))
        xtpool = es.enter_context(tc.tile_pool(name="xt", bufs=1))
        kvwpool = es.enter_context(tc.tile_pool(name="kvw", bufs=1))
        expool = es.enter_context(tc.tile_pool(name="ex", bufs=8))
        recpool = es.enter_context(tc.tile_pool(name="rec", bufs=4))
        outpool = es.enter_context(tc.tile_pool(name="outp", bufs=4))

        # first-needed transfers go ahead of everything on the Sync queue
        wq_src = wq_ap.rearrange("(u p) f -> p u f", p=P)
        wk_src = wk_ap.rearrange("(u p) f -> p u f", p=P)
        wv_src = wv_ap.rearrange("(u p) f -> p u f", p=P)
        x_src = xt_ap.rearrange("(u p) s -> p u s", p=P)

        wq_B = kvwpool.tile([P, NE, HDC], BF16, tag="wqB", name="wqB")
        wk_B = kvwpool.tile([P, NE, HDC], BF16, tag="wkB", name="wkB")
        wv_B = kvwpool.tile([P, NE, HDC], BF16, tag="wvB", name="wvB")
        xB = xtpool.tile([P, NE, S], BF16, tag="xB", name="xB")
        wq_b = [wq_B[:, u, :] for u in range(NE)]
        wk_b = [wk_B[:, u, :] for u in range(NE)]
        wv_b = [wv_B[:, u, :] for u in range(NE)]
        xb = [xB[:, u, :] for u in range(NE)]

        # gates first (3 tiny issues; the sigmoids must be ready before
        # the first QK eviction or the PSUM bank rotation stalls the PE)
        sgcol_raw = cpool.tile([P, 2], F32, tag="sgcol_raw", name="sgcol_raw")
        sgcol = cpool.tile([P, 2], F32, tag="sgcol", name="sgcol")
        nc.sync.dma_start(out=sgcol_raw[:, :],
                          in_=sq_ap.rearrange("(v p) -> p v", p=P))
        nc.scalar.activation(sgcol[:, :], sgcol_raw[:, :], Sigm)
        sgrow_raw = cpool.tile([1, HDC], F32, tag="sgrow_raw", name="sgrow_raw")
        sgrow = cpool.tile([1, HDC], F32, tag="sgrow", name="sgrow")
        nc.sync.dma_start(out=sgrow_raw[:, :], in_=sq_ap[None, :])
        nc.scalar.activation(r(sgrow[:, :]), sgrow_raw[:, :], Sigm)
        sorow_raw = cpool.tile([1, E], F32, tag="sorow_raw", name="sorow_raw")
        sorow = cpool.tile([1, E], F32, tag="sorow", name="sorow")
        nc.sync.dma_start(out=sorow_raw[:, :], in_=so_ap[None, :])
        nc.scalar.activation(r(sorow[:, :]), sorow_raw[:, :], Sigm)

        # first-chunk data, then the K/V weights (needed by ~15us),
        # then the remaining x chunks and W_out
        for half in (slice(0, 4), slice(4, 8)):
            nc.sync.dma_start(out=wq_B[:, half, :], in_=wq_src[:, half, :])
            nc.sync.dma_start(out=xB[:, half, 0:QC], in_=x_src[:, half, 0:QC])
        nc.sync.dma_start(out=wk_B[:, :, :], in_=wk_src[:, :, :])
        nc.sync.dma_start(out=wv_B[:, :, :], in_=wv_src[:, :, :])
        nc.sync.dma_start(out=xB[:, :, QC : 2 * QC],
                          in_=x_src[:, :, QC : 2 * QC])
        nc.sync.dma_start(out=xB[:, :, 2 * QC : 3 * QC],
                          in_=x_src[:, :, 2 * QC : 3 * QC])
        nc.sync.dma_start(out=xB[:, :, 3 * QC : 4 * QC],
                          in_=x_src[:, :, 3 * QC : 4 * QC])

        ones_row = cpool.tile([1, P], F32, tag="ones_row", name="ones_row")
        ones_raw = cpool.tile([1, P], F32, tag="ones_raw", name="ones_raw")
        nc.vector.memset(ones_raw[:, :], 1.0)
        nc.vector.tensor_copy(r(ones_row[:, :]), ones_raw[:, :])
        ones_b = cpool.tile([1, P], BF16, tag="ones_b", name="ones_b")
        nc.vector.tensor_copy(ones_b[:, :], ones_raw[:, :])
        ones4 = cpool.tile([P, HC], F32, tag="ones4", name="ones4")
        nc.vector.memset(ones4[:, :], 1.0)

        sqv_b = cpool.tile([P, HDC], F32, tag="sqv_b", name="sqv_b")
        so_bb = cpool.tile([P, E], BF16, tag="so_bb", name="so_bb")

        wo_b = [wopool.tile([P, E], BF16, tag=f"wob{c}", name=f"wob{c}")
                for c in range(2)]
        for c in range(2):
            nc.sync.dma_start(out=wo_b[c][:, :],
                              in_=wo_ap[c * P : (c + 1) * P, :])

        QT = [qkpool.tile([P, S], BF16, tag=f"qt{v}", name=f"qt{v}")
              for v in range(2)]
        KT = [qkpool.tile([P, S], BF16, tag=f"kt{v}", name=f"kt{v}")
              for v in range(2)]
        Vt = [vpool.tile([P, HC * 65], BF16, tag=f"v{t}", name=f"v{t}")
              for t in range(NS)]
        attnN = [anpool.tile([P, S], BF16, tag=f"an{c}", name=f"an{c}")
                 for c in range(2)]

        # ones columns of V_aug (DVE-only, no DMA dependency)
        for t in range(NS):
            nc.vector.tensor_copy(
                Vt[t][:, :].rearrange("p (h c) -> p h c", c=65)[:, :, 64],
                ones4[:, :])

        # ---------------- projection helpers ----------------
        def qk_proj(pool, tag, wname, wt, out_t, v, c, lo=0, hi=NE, ps=None):
            if ps is None:
                ps = pool.tile([P, QC], F32, tag=tag, name=f"ps{wname}{v}_{c}")
            for u in range(lo, hi):
                nc.tensor.matmul(
                    ps[:, :],
                    wt[u][:, v * P : (v + 1) * P],
                    xb[u][:, c * QC : (c + 1) * QC],
                    start=(u == 0), stop=(u == NE - 1),
                    skip_group_check=True)
            if hi == NE:
                nc.vector.tensor_scalar_mul(
                    out_t[v][:, c * QC : (c + 1) * QC], ps[:, :],
                    sgcol[:, v : v + 1])
            return ps

        def v_proj(pool, tag, t, lo=0, hi=NE, ps=None):
            if ps is None:
                ps = pool.tile([P, QC], F32, tag=tag, name=f"psv{t}")
            for u in range(lo, hi):
                nc.tensor.matmul(
                    ps[:, 0:HDC],
                    xb[u][:, t * P : (t + 1) * P],
                    wv_b[u][:, :],
                    start=(u == 0), stop=(u == NE - 1),
                    skip_group_check=True)
            if hi == NE:
                nc.vector.tensor_mul(
                    Vt[t][:, :].rearrange("p (h c) -> p h c", c=65)[:, :, 0:64],
                    ps[:, 0:HDC].rearrange("p (h d) -> p h d", d=D),
                    sqv_b[:, :].rearrange("p (h d) -> p h d", d=D))
            return ps

        # ---------------- phase 1: QKV projections ----------------
        # chunk 3's K and V projections are deferred into the attention
        # loop (they are only consumed by attention chunk 3, and they
        # give the late, filler-starved slots real K=128 PE work).
        # ALL stream-side PSUM (projections, scores, outproj, broadcasts,
        # deferred projections) shares ONE 4-deep rotation on banks 0-3;
        # the attention accumulators get fresh banks 4-7 that phase 1
        # never touches.  The last projection tiles are >=4 allocations
        # before the first scores tile, so attention starts with no WAR
        # wait on phase 1 (a PE gap at the seam re-arms the HAM throttle
        # and costs 10-20us at half clock).
        scps = es.enter_context(tc.tile_pool(name="ps_sc", bufs=4,
                                             space="PSUM"))
        atps = es.enter_context(tc.tile_pool(name="ps_at", bufs=4,
                                             space="PSUM"))

        for c in range(NQ):
            for wname, wt, out_t in (("q", wq_b, QT), ("k", wk_b, KT)):
                if c >= 2 and wname == "k":
                    continue  # deferred into phase 2
                for v in range(2):
                    qk_proj(scps, "sc", wname, wt, out_t, v, c)

            if c == 0:
                # gate broadcasts over partitions via PE (K=1 matmuls);
                # placed after chunk 0's projections so the PE doesn't
                # head-of-line block on the gate DMAs
                bq = scps.tile([P, QC], F32, tag="sc", name="bq")
                nc.tensor.matmul(bq[:, 0:HDC], r(ones_row[:, :]),
                                 r(sgrow[:, :]), start=True, stop=True)
                nc.vector.tensor_copy(sqv_b[:, :], bq[:, 0:HDC])
                for cc in range(2):
                    bo = scps.tile([P, QC], F32, tag="sc", name=f"bo{cc}")
                    nc.tensor.matmul(bo[:, :], r(ones_row[:, :]),
                                     r(sorow[:, cc * QC : (cc + 1) * QC]),
                                     start=True, stop=True)
                    nc.vector.tensor_copy(
                        so_bb[:, cc * QC : (cc + 1) * QC], bo[:, :])
                for cc in range(2):
                    nc.vector.tensor_mul(wo_b[cc][:, :], wo_b[cc][:, :],
                                         so_bb[:, :])

            for t in range(4 * c, 4 * c + 4):
                if c >= 2:
                    continue  # deferred into phase 2
                v_proj(scps, "sc", t)

        # ---------------- phase 2: attention + output projection ----------------
        # queued PE work (outproj matmuls, normalizations, deferred
        # chunk-3 projections), popped one item per k-tile slot so the
        # PE never runs dry mid-attention
        # norms go through a priority queue: each must pop within ~2
        # blocks of creation or the attn-accumulator bank rotation wraps
        # onto an un-normalized accumulator and the PE FIFO deadlocks
        # (the norm's broadcast matmul would sit behind the very matmul
        # waiting for it)
        prio = deque()
        fillers = deque()

        def pop_filler(reserve=0):
            if prio:
                prio.popleft()()
            elif len(fillers) > reserve:
                fillers.popleft()()

        def emit_outproj(i):
            for t in range(4 * i, 4 * i + 4):
                ot = outpool.tile([P, E], BF16, tag="ot", name=f"ot{t}")
                for eh in range(2):
                    def op_item(t=t, eh=eh, ot=ot):
                        po = scps.tile([P, QC], F32, tag="sc",
                                       name=f"po{t}_{eh}")
                        for c in range(2):
                            nc.tensor.matmul(
                                po[:, :],
                                attnN[c][:, t * P : (t + 1) * P],
                                wo_b[c][:, eh * QC : (eh + 1) * QC],
                                start=(c == 0), stop=(c == 1),
                                skip_group_check=True)
                        nc.vector.tensor_copy(
                            ot[:, eh * QC : (eh + 1) * QC], po[:, :])
                        if eh == 1:
                            nc.sync.dma_start(
                                out=out_ap[t * P : (t + 1) * P, :],
                                in_=ot[:, :])
                    fillers.append(op_item)

        def defer_k(c):
            for v in range(2):
                state = {}

                def ka(v=v, c=c, state=state):
                    state["ps"] = qk_proj(scps, "sc", "k", wk_b, KT, v, c,
                                          0, 4)

                def kb(v=v, c=c, state=state):
                    qk_proj(scps, "sc", "k", wk_b, KT, v, c, 4, NE,
                            state["ps"])
                fillers.append(ka)
                fillers.append(kb)

        def defer_v(ts):
            for t in ts:
                state = {}

                def va(t=t, state=state):
                    state["ps"] = v_proj(scps, "sc", t, 0, 4)

                def vb(t=t, state=state):
                    v_proj(scps, "sc", t, 4, NE, state["ps"])
                fillers.append(va)
                fillers.append(vb)

        def emit_deferred_proj():
            # chunks 2+3's K and V projections, split into half-items so
            # a single pop doesn't starve the exp pipeline; ordered by
            # first-use time (V-c2 at attn (2,0) j=8, K-c2 at scores
            # (2,0) j=8, the rest in chunk 3)
            defer_v(range(8, 12))
            defer_k(2)
            defer_k(3)
            defer_v(range(12, 16))

        # the deferred chunk-3 projections are ready to run from the very
        # first attention slot (their inputs landed with the phase-1
        # DMAs) -- emitting them here keeps the otherwise filler-less
        # chunks 0-1 dense so the HAM clock gate never re-arms early
        emit_deferred_proj()

        for i in range(NQ):
            for p in range(2):
                # head pair (2p, 2p+1): even head on PE rows 0-63, odd
                # head on rows 64-127 -- the two K=64 score matmuls
                # row-tile into disjoint quadrant rows and run
                # concurrently in the array
                kt, qt = KT[p], QT[p]
                njs = 4 * i + 4
                accA = atps.tile([65, QC], F32, tag="at", name=f"aA{i}{p}")
                accB = atps.tile([65, QC], F32, tag="at", name=f"aB{i}{p}")
                hA, hB = 2 * p, 2 * p + 1
                exs = {}

                def attn_mm(j, accA=accA, accB=accB, njs=njs, exs=exs,
                            hA=hA, hB=hB):
                    exA, exB, off, wdt = exs.pop(j)
                    nc.tensor.matmul(
                        accA[:, off : off + wdt],
                        Vt[j][:, hA * 65 : hA * 65 + 65],
                        exA[:, 0:wdt], start=(j == 0),
                        stop=(j == njs - 1), skip_group_check=True)
                    nc.tensor.matmul(
                        accB[:, off : off + wdt],
                        Vt[j][:, hB * 65 : hB * 65 + 65],
                        exB[:, 0:wdt], start=(j == 0),
                        stop=(j == njs - 1), skip_group_check=True)

                for j in range(njs):
                    diag = j >= 4 * i
                    off = max(0, j * P - i * QC)
                    wdt = QC - off
                    spA = scps.tile([P, QC], F32, tag="sc",
                                    name=f"sA{i}_{p}_{j}")
                    spB = scps.tile([P, QC], F32, tag="sc",
                                    name=f"sB{i}_{p}_{j}")
                    exA = expool.tile([P, QC], BF16, tag="ex",
                                      name=f"eA{i}_{p}_{j}")
                    exB = expool.tile([P, QC], BF16, tag="ex",
                                      name=f"eB{i}_{p}_{j}")
                    nc.tensor.matmul(
                        spA[:, 0:wdt],
                        kt[0:D, j * P : (j + 1) * P],
                        qt[0:D, i * QC + off : i * QC + off + wdt],
                        start=True, stop=True)
                    nc.tensor.matmul(
                        spB[:, 0:wdt],
                        kt[D : 2 * D, j * P : (j + 1) * P],
                        qt[D : 2 * D, i * QC + off : i * QC + off + wdt],
                        start=True, stop=True)
                    nc.scalar.activation(exA[:, 0:wdt], spA[:, 0:wdt],
                                         Exp, scale=0.125)
                    nc.scalar.activation(exB[:, 0:wdt], spB[:, 0:wdt],
                                         Exp, scale=0.125)
                    if diag:
                        # zero the q<k half of the leading [128,128]
                        # window on GPSIMD (otherwise idle)
                        for exx in (exA, exB):
                            nc.gpsimd.affine_select(
                                out=exx[:, 0:P], in_=exx[:, 0:P],
                                compare_op=mybir.AluOpType.is_ge,
                                fill=0.0, base=0,
                                pattern=[[1, P]], channel_multiplier=-1)
                    exs[j] = (exA, exB, off, wdt)
                    if j >= LAG:
                        attn_mm(j - LAG)
                    if j >= 2:
                        # one pop per slot: filler supply (72 items) is
                        # matched to the eligible slots (64), so the
                        # queue never runs dry mid-attention (an
                        # underfilled stretch re-arms the HAM throttle).
                        # In the very last block hold 4 items in reserve
                        # -- they pop right after the final attention
                        # matmuls and cover the normalization chain's
                        # DVE latency, so the PE has no gap going into
                        # the tail
                        reserve = 4 if (i == NQ - 1 and p == 1) else 0
                        pop_filler(reserve)
                attn_mm(njs - 2)
                attn_mm(njs - 1)

                # stage the L rows down to partition 0, approx-invert the
                # [1, QC] rows, round to bf16; the deferred norm
                # broadcasts 1/L on the PE (bf16 K=1 matmul) and applies
                # it on DVE
                lrA = recpool.tile([1, QC], F32, tag="lr", name=f"lA{i}{p}")
                lrB = recpool.tile([1, QC], F32, tag="lr", name=f"lB{i}{p}")
                nc.vector.tensor_copy(lrA[:, :], accA[64:65, :])
                nc.vector.tensor_copy(lrB[:, :], accB[64:65, :])
                liA = recpool.tile([1, QC], F32, tag="li", name=f"iA{i}{p}")
                liB = recpool.tile([1, QC], F32, tag="li", name=f"iB{i}{p}")
                nc.vector.reciprocal_approx_fast(liA[:, :], lrA[:, :])
                nc.vector.reciprocal_approx_fast(liB[:, :], lrB[:, :])
                lbA = recpool.tile([1, QC], BF16, tag="lb", name=f"bA{i}{p}")
                lbB = recpool.tile([1, QC], BF16, tag="lb", name=f"bB{i}{p}")
                nc.vector.tensor_copy(lbA[:, :], liA[:, :])
                nc.vector.tensor_copy(lbB[:, :], liB[:, :])

                def norm(acc, lb, hr, p=p, i=i):
                    def run(acc=acc, lb=lb, hr=hr, p=p, i=i):
                        bc = scps.tile([P, QC], F32, tag="sc",
                                       name=f"bc{i}_{p}_{hr}")
                        nc.tensor.matmul(bc[:, :], ones_b[:, :], lb[:, :],
                                         start=True, stop=True)
                        bcs = recpool.tile([64, QC], F32, tag="bcs",
                                           name=f"bs{i}_{p}_{hr}")
                        # ACT is half-idle in attention; staging the
                        # broadcast there keeps the DVE queue short
                        nc.scalar.copy(bcs[:, :], bc[0:64, :])
                        nc.vector.tensor_mul(
                            attnN[p][hr : hr + D, i * QC : (i + 1) * QC],
                            acc[0:64, :], bcs[:, :])
                    return run

                prio.append(norm(accA, lbA, 0))
                prio.append(norm(accB, lbB, 64))

            emit_outproj(i)

        while prio or fillers:
            pop_filler()


_NC_CACHE = {}


def _get_nc():
    if "nc" in _NC_CACHE:
        return _NC_CACHE["nc"]
    nc = bacc.Bacc("TRN2", target_bir_lowering=False, debug=False,
                   enable_asserts=False, num_devices=8)
    xt_h = nc.dram_tensor("xt", [E, S], BF16, kind="ExternalInput")
    wq_h = nc.dram_tensor("wq", [E, HDC], BF16, kind="ExternalInput")
    wk_h = nc.dram_tensor("wk", [E, HDC], BF16, kind="ExternalInput")
    wv_h = nc.dram_tensor("wv", [E, HDC], BF16, kind="ExternalInput")
    wo_h = nc.dram_tensor("wo", [HDC, E], BF16, kind="ExternalInput")
    sq_h = nc.dram_tensor("sq", [HDC], F32, kind="ExternalInput")
    so_h = nc.dram_tensor("so", [E], F32, kind="ExternalInput")
    out_h = nc.dram_tensor("out", [S, E], BF16, kind="ExternalOutput")
    with tile.TileContext(nc) as tc:
        build(tc, out_h.ap(), xt_h.ap(), wq_h.ap(), wk_h.ap(), wv_h.ap(),
              wo_h.ap(), sq_h.ap(), so_h.ap())
    nc.compile()
    _NC_CACHE["nc"] = nc
    return nc


def make_in_maps(x, W_qkv, W_out, scale_qkv, scale_out, mask=None):
    import ml_dtypes
    BF = ml_dtypes.bfloat16
    in_maps = []
    sq_full = np.ascontiguousarray(scale_qkv, np.float32).reshape(H, D)
    xts = [np.ascontiguousarray(
        np.asarray(x[b], np.float32).T.astype(BF)) for b in range(B)]
    for b in range(B):
        for g in range(4):
            hs = slice(HC * g, HC * g + HC)
            in_maps.append({
                "xt": xts[b],
                "wq": np.ascontiguousarray(
                    W_qkv[:, 0, hs, :], np.float32).reshape(E, HDC).astype(BF),
                "wk": np.ascontiguousarray(
                    W_qkv[:, 1, hs, :], np.float32).reshape(E, HDC).astype(BF),
                "wv": np.ascontiguousarray(
                    W_qkv[:, 2, hs, :], np.float32).reshape(E, HDC).astype(BF),
                "wo": np.ascontiguousarray(
                    W_out[hs], np.float32).reshape(HDC, E).astype(BF),
                "sq": np.ascontiguousarray(sq_full[hs], np.float32).reshape(HDC),
                "so": np.ascontiguousarray(scale_out, np.float32),
            })
    return in_maps


def kernel(x, W_qkv, W_out, scale_qkv, scale_out, mask=None, _runner_kwargs=None):
    nc = _get_nc()
    in_maps = make_in_maps(x, W_qkv, W_out, scale_qkv, scale_out)
    kw = _runner_kwargs or {}
    res = run_bass_kernel_spmd(nc, in_maps, core_ids=list(range(8)), **kw)
    if _runner_kwargs is not None:
        kernel.last_results = res
    outs = [np.asarray(res.results[i]["out"], np.float32) for i in range(8)]
    full = np.empty((B, S, E), np.float32)
    for b in range(B):
        full[b] = outs[4 * b] + outs[4 * b + 1] + outs[4 * b + 2] + outs[4 * b + 3]
    return full


if __name__ == "__main__":
    rng = np.random.default_rng(0)
    inputs = {
        "x": rng.standard_normal((B, S, E)).astype(np.float32),
        "W_qkv": (rng.standard_normal((E, 3, H, D)).astype(np.float32) * E ** -0.5),
        "W_out": (rng.standard_normal((H, D, E)).astype(np.float32)
                  * (H * D) ** -0.5),
        "scale_qkv": (rng.standard_normal(E).astype(np.float32) * 0.02 + 1.0),
        "scale_out": (rng.standard_normal(E).astype(np.float32) * 0.02 + 1.0),
        "mask": np.tril(np.ones((S, S), bool)),
    }
    out = kernel(**inputs)
    print("kernel ran, out shape", out.shape, out.dtype)


# revision 56
# speedup vs baseline: 1.0089x; 1.0012x over previous
"""Trainium2 Bass kernel for ConstantModulationAttention.

Reference computation (B=2, S=2048, E=1024, H=16, D=64):
    sq  = sigmoid(scale_qkv).reshape(H, D)
    so  = sigmoid(scale_out)
    qkv = einsum('bse,eqhd->bqshd', x, W_qkv) * sq
    Q, K, V = qkv[:,0], qkv[:,1], qkv[:,2]
    scores  = einsum('bshd,bthd->bhst', Q, K) / 8
    w       = softmax(where(causal, scores, -inf), axis=-1)
    attn    = einsum('bhst,bthd->bshd', w, V)
    out     = einsum('bshd,hde->bse', attn, W_out) * so

Sharding: 8 cores = 2 (batch) x 4 (head groups of 4 heads).  Each core
computes a partial output over its 4 heads; the host sums the 4 partials
per batch (the tensor-parallel reduce) and stacks the 2 batches.  The
host hands each core x^T in bf16 (layout marshalling, part of sharding).

Per-core device program (all PE matmuls bf16 except tiny f32r helpers):
  xT    [E, S]   bf16, DMA'd straight into SBUF (no on-chip casts)
  QT,KT [HD, S]  = (W^T x^T), gated per-partition (hd) on eviction
  V     [S, HD]  natural layout, gated, augmented with a ones column per
                 head (V_aug [S, 65]) so the attention matmul's 65th row
                 yields the softmax denominator L
  scores: heads processed in PAIRS -- even head on PE rows 0-63, odd
                 head on rows 64-127 (K=64 row-tiled matmuls, concurrent
                 in the array) so the pair streams in the time of one
  exp on ACT (scale=1/8 folds the 1/sqrt(D)), bf16 out; ACT does exp ONLY
  attnT [65, q]  = V_aug^T @ exp accumulated over k; row 64 = L
  attnN [hd, s]  = attnT[0:64] * (1/L) (bf16 PE broadcast of the
                 [1,QC] approx-reciprocal, applied on DVE)
  out   [s, e]   = attnN^T @ (W_out * so), bf16 partial DMA'd out

ALL stream-side PSUM (projections, scores, outproj, broadcasts) shares
ONE 5-deep rotating pool, so there is no pool barrier between the
projection and attention phases and no PE gap at the seam (a PE gap of
>~2us re-arms the HAM throttle and costs 10-20us at half clock).
Outproj matmuls, normalizations, and the deferred chunk-3 K/V
projections are queued and popped one per k-tile slot inside the
attention loop, so the PE stays dense through the late, filler-starved
chunks as well.
"""

from collections import deque
from contextlib import ExitStack

import numpy as np

import concourse.bass as bass  # noqa: F401
import concourse.tile as tile
from concourse import bacc, mybir
from concourse.bass_utils import run_bass_kernel_spmd

F32 = mybir.dt.float32
F32R = mybir.dt.float32r
BF16 = mybir.dt.bfloat16

B, S, E, H, D = 2, 2048, 1024, 16, 64
P = 128
HC = H // 4          # 4 heads per core
HDC = HC * D         # 256 per-core head-dim features
NE = E // P          # 8 e-tiles
NS = S // P          # 16 s-tiles
QC = 512             # q-chunk width
NQ = S // QC         # 4 q-chunks
LAG = 2              # attn matmul trails the scores by LAG k-tiles


def r(ap):
    """bitcast an fp32 AP to fp32r for full-rate PE matmuls."""
    return ap.bitcast(F32R)


def build(tc, out_ap, xt_ap, wq_ap, wk_ap, wv_ap, wo_ap, sq_ap, so_ap):
    nc = tc.nc
    Exp = mybir.ActivationFunctionType.Exp
    Sigm = mybir.ActivationFunctionType.Sigmoid

    with ExitStack() as es:
        # ---------------- pools (no phase scoping) ----------------
        cpool = es.enter_context(tc.tile_pool(name="const", bufs=1))
        wopool = es.enter_context(tc.tile_pool(name="wo", bufs=1))
        qkpool = es.enter_context(tc.tile_pool(name="qk", bufs=1))
        vpool = es.enter_context(tc.tile_pool(name="vp", bufs=1))
        anpool = es.enter_context(tc.tile_pool(name="an", bufs=1))
        xtpool = es.enter_context(tc.tile_pool(name="xt", bufs=1))
        kvwpool = es.enter_context(tc.tile_pool(name="kvw", bufs=1))
        expool = es.enter_context(tc.tile_pool(name="ex", bufs=8))
        recpool = es.enter_context(tc.tile_pool(name="rec", bufs=4))
        outpool = es.enter_context(tc.tile_pool(name="outp", bufs=4))

        # first-needed transfers go ahead of everything on the Sync queue
        wq_src = wq_ap.rearrange("(u p) f -> p u f", p=P)
        wk_src = wk_ap.rearrange("(u p) f -> p u f", p=P)
        wv_src = wv_ap.rearrange("(u p) f -> p u f", p=P)
        x_src = xt_ap.rearrange("(u p) s -> p u s", p=P)

        wq_B = kvwpool.tile([P, NE, HDC], BF16, tag="wqB", name="wqB")
        wk_B = kvwpool.tile([P, NE, HDC], BF16, tag="wkB", name="wkB")
        wv_B = kvwpool.tile([P, NE, HDC], BF16, tag="wvB", name="wvB")
        xB = xtpool.tile([P, NE, S], BF16, tag="xB", name="xB")
        wq_b = [wq_B[:, u, :] for u in range(NE)]
        wk_b = [wk_B[:, u, :] for u in range(NE)]
        wv_b = [wv_B[:, u, :] for u in range(NE)]
        xb = [xB[:, u, :] for u in range(NE)]

        # gates first (3 tiny issues; the sigmoids must be ready before
        # the first QK eviction or the PSUM bank rotation stalls the PE)
        sgcol_raw = cpool.tile([P, 2], F32, tag="sgcol_raw", name="sgcol_raw")
        sgcol = cpool.tile([P, 2], F32, tag="sgcol", name="sgcol")
        nc.sync.dma_start(out=sgcol_raw[:, :],
                          in_=sq_ap.rearrange("(v p) -> p v", p=P))
        nc.scalar.activation(sgcol[:, :], sgcol_raw[:, :], Sigm)
        sgrow_raw = cpool.tile([1, HDC], F32, tag="sgrow_raw", name="sgrow_raw")
        sgrow = cpool.tile([1, HDC], F32, tag="sgrow", name="sgrow")
        nc.sync.dma_start(out=sgrow_raw[:, :], in_=sq_ap[None, :])
        nc.scalar.activation(r(sgrow[:, :]), sgrow_raw[:, :], Sigm)
        sorow_raw = cpool.tile([1, E], F32, tag="sorow_raw", name="sorow_raw")
        sorow = cpool.tile([1, E], F32, tag="sorow", name="sorow")
        nc.sync.dma_start(out=sorow_raw[:, :], in_=so_ap[None, :])
        nc.scalar.activation(r(sorow[:, :]), sorow_raw[:, :], Sigm)

        # first-chunk data, then the K/V weights (needed by ~15us),
        # then the remaining x chunks and W_out
        for half in (slice(0, 4), slice(4, 8)):
            nc.sync.dma_start(out=wq_B[:, half, :], in_=wq_src[:, half, :])
            nc.sync.dma_start(out=xB[:, half, 0:QC], in_=x_src[:, half, 0:QC])
        nc.sync.dma_start(out=wk_B[:, :, :], in_=wk_src[:, :, :])
        nc.sync.dma_start(out=wv_B[:, :, :], in_=wv_src[:, :, :])
        nc.sync.dma_start(out=xB[:, :, QC : 2 * QC],
                          in_=x_src[:, :, QC : 2 * QC])
        nc.sync.dma_start(out=xB[:, :, 2 * QC : 3 * QC],
                          in_=x_src[:, :, 2 * QC : 3 * QC])
        nc.sync.dma_start(out=xB[:, :, 3 * QC : 4 * QC],
                          in_=x_src[:, :, 3 * QC : 4 * QC])

        ones_row = cpool.tile([1, P], F32, tag="ones_row", name="ones_row")
        ones_raw = cpool.tile([1, P], F32, tag="ones_raw", name="ones_raw")
        nc.vector.memset(ones_raw[:, :], 1.0)
        nc.vector.tensor_copy(r(ones_row[:, :]), ones_raw[:, :])
        ones_b = cpool.tile([1, P], BF16, tag="ones_b", name="ones_b")
        nc.vector.tensor_copy(ones_b[:, :], ones_raw[:, :])
        ones4 = cpool.tile([P, HC], F32, tag="ones4", name="ones4")
        nc.vector.memset(ones4[:, :], 1.0)

        sqv_b = cpool.tile([P, HDC], F32, tag="sqv_b", name="sqv_b")
        so_bb = cpool.tile([P, E], BF16, tag="so_bb", name="so_bb")

        wo_b = [wopool.tile([P, E], BF16, tag=f"wob{c}", name=f"wob{c}")
                for c in range(2)]
        for c in range(2):
            nc.sync.dma_start(out=wo_b[c][:, :],
                              in_=wo_ap[c * P : (c + 1) * P, :])

        QT = [qkpool.tile([P, S], BF16, tag=f"qt{v}", name=f"qt{v}")
              for v in range(2)]
        KT = [qkpool.tile([P, S], BF16, tag=f"kt{v}", name=f"kt{v}")
              for v in range(2)]
        Vt = [vpool.tile([P, HC * 65], BF16, tag=f"v{t}", name=f"v{t}")
              for t in range(NS)]
        attnN = [anpool.tile([P, S], BF16, tag=f"an{c}", name=f"an{c}")
                 for c in range(2)]

        # ones columns of V_aug (DVE-only, no DMA dependency)
        for t in range(NS):
            nc.vector.tensor_copy(
                Vt[t][:, :].rearrange("p (h c) -> p h c", c=65)[:, :, 64],
                ones4[:, :])

        # ---------------- projection helpers ----------------
        def qk_proj(pool, tag, wname, wt, out_t, v, c, lo=0, hi=NE, ps=None):
            if ps is None:
                ps = pool.tile([P, QC], F32, tag=tag, name=f"ps{wname}{v}_{c}")
            for u in range(lo, hi):
                nc.tensor.matmul(
                    ps[:, :],
                    wt[u][:, v * P : (v + 1) * P],
                    xb[u][:, c * QC : (c + 1) * QC],
                    start=(u == 0), stop=(u == NE - 1),
                    skip_group_check=True)
            if hi == NE:
                nc.vector.tensor_scalar_mul(
                    out_t[v][:, c * QC : (c + 1) * QC], ps[:, :],
                    sgcol[:, v : v + 1])
            return ps

        def v_proj(pool, tag, t, lo=0, hi=NE, ps=None):
            if ps is None:
                ps = pool.tile([P, QC], F32, tag=tag, name=f"psv{t}")
            for u in range(lo, hi):
                nc.tensor.matmul(
                    ps[:, 0:HDC],
                    xb[u][:, t * P : (t + 1) * P],
                    wv_b[u][:, :],
                    start=(u == 0), stop=(u == NE - 1),
                    skip_group_check=True)
            if hi == NE:
                nc.vector.tensor_mul(
                    Vt[t][:, :].rearrange("p (h c) -> p h c", c=65)[:, :, 0:64],
                    ps[:, 0:HDC].rearrange("p (h d) -> p h d", d=D),
                    sqv_b[:, :].rearrange("p (h d) -> p h d", d=D))
            return ps

        # ---------------- phase 1: QKV projections ----------------
        # chunk 3's K and V projections are deferred into the attention
        # loop (they are only consumed by attention chunk 3, and they
        # give the late, filler-starved slots real K=128 PE work).
        # ALL stream-side PSUM (projections, scores, outproj, broadcasts,
        # deferred projections) shares ONE 4-deep rotation on banks 0-3;
        # the attention accumulators get fresh banks 4-7 that phase 1
        # never touches.  The last projection tiles are >=4 allocations
        # before the first scores tile, so attention starts with no WAR
        # wait on phase 1 (a PE gap at the seam re-arms the HAM throttle
        # and costs 10-20us at half clock).
        scps = es.enter_context(tc.tile_pool(name="ps_sc", bufs=4,
                                             space="PSUM"))
        atps = es.enter_context(tc.tile_pool(name="ps_at", bufs=4,
                                             space="PSUM"))

        for c in range(NQ):
            for wname, wt, out_t in (("q", wq_b, QT), ("k", wk_b, KT)):
                if c >= 2 and wname == "k":
                    continue  # deferred into phase 2
                for v in range(2):
                    qk_proj(scps, "sc", wname, wt, out_t, v, c)

            if c == 0:
                # gate broadcasts over partitions via PE (K=1 matmuls);
                # placed after chunk 0's projections so the PE doesn't
                # head-of-line block on the gate DMAs
                bq = scps.tile([P, QC], F32, tag="sc", name="bq")
                nc.tensor.matmul(bq[:, 0:HDC], r(ones_row[:, :]),
                                 r(sgrow[:, :]), start=True, stop=True)
                nc.vector.tensor_copy(sqv_b[:, :], bq[:, 0:HDC])
                for cc in range(2):
                    bo = scps.tile([P, QC], F32, tag="sc", name=f"bo{cc}")
                    nc.tensor.matmul(bo[:, :], r(ones_row[:, :]),
                                     r(sorow[:, cc * QC : (cc + 1) * QC]),
                                     start=True, stop=True)
                    nc.vector.tensor_copy(
                        so_bb[:, cc * QC : (cc + 1) * QC], bo[:, :])
                for cc in range(2):
                    nc.vector.tensor_mul(wo_b[cc][:, :], wo_b[cc][:, :],
                                         so_bb[:, :])

            for t in range(4 * c, 4 * c + 4):
                if c >= 2:
                    continue  # deferred into phase 2
                v_proj(scps, "sc", t)

        # ---------------- phase 2: attention + output projection ----------------
        # queued PE work (outproj matmuls, normalizations, deferred
        # chunk-3 projections), popped one item per k-tile slot so the
        # PE never runs dry mid-attention
        # norms go through a priority queue: each must pop within ~2
        # blocks of creation or the attn-accumulator bank rotation wraps
        # onto an un-normalized accumulator and the PE FIFO deadlocks
        # (the norm's broadcast matmul would sit behind the very matmul
        # waiting for it)
        prio = deque()
        fillers = deque()

        def pop_filler(reserve=0):
            if prio:
                prio.popleft()()
            elif len(fillers) > reserve:
                fillers.popleft()()

        def emit_outproj(i):
            for t in range(4 * i, 4 * i + 4):
                ot = outpool.tile([P, E], BF16, tag="ot", name=f"ot{t}")
                for eh in range(2):
                    def op_item(t=t, eh=eh, ot=ot):
                        po = scps.tile([P, QC], F32, tag="sc",
                                       name=f"po{t}_{eh}")
                        for c in range(2):
                            nc.tensor.matmul(
                                po[:, :],
                                attnN[c][:, t * P : (t + 1) * P],
                                wo_b[c][:, eh * QC : (eh + 1) * QC],
                                start=(c == 0), stop=(c == 1),
                                skip_group_check=True)
                        nc.vector.tensor_copy(
                            ot[:, eh * QC : (eh + 1) * QC], po[:, :])
                        if eh == 1:
                            nc.sync.dma_start(
                                out=out_ap[t * P : (t + 1) * P, :],
                                in_=ot[:, :])
                    fillers.append(op_item)

        def defer_k(c):
            for v in range(2):
                state = {}

                def ka(v=v, c=c, state=state):
                    state["ps"] = qk_proj(scps, "sc", "k", wk_b, KT, v, c,
                                          0, 4)

                def kb(v=v, c=c, state=state):
                    qk_proj(scps, "sc", "k", wk_b, KT, v, c, 4, NE,
                            state["ps"])
                fillers.append(ka)
                fillers.append(kb)

        def defer_v(ts):
            for t in ts:
                state = {}

                def va(t=t, state=state):
                    state["ps"] = v_proj(scps, "sc", t, 0, 4)

                def vb(t=t, state=state):
                    v_proj(scps, "sc", t, 4, NE, state["ps"])
                fillers.append(va)
                fillers.append(vb)

        def emit_deferred_proj():
            # chunks 2+3's K and V projections, split into half-items so
            # a single pop doesn't starve the exp pipeline; ordered by
            # first-use time (V-c2 at attn (2,0) j=8, K-c2 at scores
            # (2,0) j=8, the rest in chunk 3)
            defer_v(range(8, 12))
            defer_k(2)
            defer_k(3)
            defer_v(range(12, 16))

        # the deferred chunk-3 projections are ready to run from the very
        # first attention slot (their inputs landed with the phase-1
        # DMAs) -- emitting them here keeps the otherwise filler-less
        # chunks 0-1 dense so the HAM clock gate never re-arms early
        emit_deferred_proj()

        # chunk 0/1 blocks interleave: the short, sparse chunk-0 blocks
        # (4 small-width slots each) are sandwiched between chunk 1's
        # denser full-width blocks, keeping the PE duty cycle over any
        # HAM window high enough that the clock gate never re-arms in
        # early attention
        BLOCKS = [(0, 0), (1, 0), (0, 1), (1, 1),
                  (2, 0), (2, 1), (3, 0), (3, 1)]
        for i, p in BLOCKS:
                # head pair (2p, 2p+1): even head on PE rows 0-63, odd
                # head on rows 64-127 -- the two K=64 score matmuls
                # row-tile into disjoint quadrant rows and run
                # concurrently in the array
                kt, qt = KT[p], QT[p]
                njs = 4 * i + 4
                accA = atps.tile([65, QC], F32, tag="at", name=f"aA{i}{p}")
                accB = atps.tile([65, QC], F32, tag="at", name=f"aB{i}{p}")
                hA, hB = 2 * p, 2 * p + 1
                exs = {}

                def attn_mm(j, accA=accA, accB=accB, njs=njs, exs=exs,
                            hA=hA, hB=hB):
                    exA, exB, off, wdt = exs.pop(j)
                    nc.tensor.matmul(
                        accA[:, off : off + wdt],
                        Vt[j][:, hA * 65 : hA * 65 + 65],
                        exA[:, 0:wdt], start=(j == 0),
                        stop=(j == njs - 1), skip_group_check=True)
                    nc.tensor.matmul(
                        accB[:, off : off + wdt],
                        Vt[j][:, hB * 65 : hB * 65 + 65],
                        exB[:, 0:wdt], start=(j == 0),
                        stop=(j == njs - 1), skip_group_check=True)

                for j in range(njs):
                    diag = j >= 4 * i
                    off = max(0, j * P - i * QC)
                    wdt = QC - off
                    spA = scps.tile([P, QC], F32, tag="sc",
                                    name=f"sA{i}_{p}_{j}")
                    spB = scps.tile([P, QC], F32, tag="sc",
                                    name=f"sB{i}_{p}_{j}")
                    exA = expool.tile([P, QC], BF16, tag="ex",
                                      name=f"eA{i}_{p}_{j}")
                    exB = expool.tile([P, QC], BF16, tag="ex",
                                      name=f"eB{i}_{p}_{j}")
                    nc.tensor.matmul(
                        spA[:, 0:wdt],
                        kt[0:D, j * P : (j + 1) * P],
                        qt[0:D, i * QC + off : i * QC + off + wdt],
                        start=True, stop=True)
                    nc.tensor.matmul(
                        spB[:, 0:wdt],
                        kt[D : 2 * D, j * P : (j + 1) * P],
                        qt[D : 2 * D, i * QC + off : i * QC + off + wdt],
                        start=True, stop=True)
                    nc.scalar.activation(exA[:, 0:wdt], spA[:, 0:wdt],
                                         Exp, scale=0.125)
                    nc.scalar.activation(exB[:, 0:wdt], spB[:, 0:wdt],
                                         Exp, scale=0.125)
                    if diag:
                        # zero the q<k half of the leading [128,128]
                        # window on GPSIMD (otherwise idle)
                        for exx in (exA, exB):
                            nc.gpsimd.affine_select(
                                out=exx[:, 0:P], in_=exx[:, 0:P],
                                compare_op=mybir.AluOpType.is_ge,
                                fill=0.0, base=0,
                                pattern=[[1, P]], channel_multiplier=-1)
                    exs[j] = (exA, exB, off, wdt)
                    if j >= LAG:
                        attn_mm(j - LAG)
                    if j >= 2:
                        # one pop per slot: filler supply (72 items) is
                        # matched to the eligible slots (64), so the
                        # queue never runs dry mid-attention (an
                        # underfilled stretch re-arms the HAM throttle).
                        # In the very last block hold 4 items in reserve
                        # -- they pop right after the final attention
                        # matmuls and cover the normalization chain's
                        # DVE latency, so the PE has no gap going into
                        # the tail
                        reserve = 4 if (i == NQ - 1 and p == 1) else 0
                        pop_filler(reserve)
                attn_mm(njs - 2)
                attn_mm(njs - 1)

                # stage the L rows down to partition 0, approx-invert the
                # [1, QC] rows, round to bf16; the deferred norm
                # broadcasts 1/L on the PE (bf16 K=1 matmul) and applies
                # it on DVE
                lrA = recpool.tile([1, QC], F32, tag="lr", name=f"lA{i}{p}")
                lrB = recpool.tile([1, QC], F32, tag="lr", name=f"lB{i}{p}")
                nc.vector.tensor_copy(lrA[:, :], accA[64:65, :])
                nc.vector.tensor_copy(lrB[:, :], accB[64:65, :])
                liA = recpool.tile([1, QC], F32, tag="li", name=f"iA{i}{p}")
                liB = recpool.tile([1, QC], F32, tag="li", name=f"iB{i}{p}")
                nc.vector.reciprocal_approx_fast(liA[:, :], lrA[:, :])
                nc.vector.reciprocal_approx_fast(liB[:, :], lrB[:, :])

                def norm(acc, li, hr, p=p, i=i):
                    # 1/L spread over partitions on the idle GPSIMD (no
                    # PE matmul, no LDW in the stream, f32 throughout),
                    # applied on DVE
                    def run(acc=acc, li=li, hr=hr, p=p, i=i):
                        bcs = recpool.tile([64, QC], F32, tag="bcs",
                                           name=f"bs{i}_{p}_{hr}")
                        nc.gpsimd.partition_broadcast(bcs[:, :], li[:, :])
                        nc.vector.tensor_mul(
                            attnN[p][hr : hr + D, i * QC : (i + 1) * QC],
                            acc[0:64, :], bcs[:, :])
                    return run

                prio.append(norm(accA, liA, 0))
                prio.append(norm(accB, liB, 64))

                # outproj for a chunk is emitted once both its pairs'
                # norms are queued (chunks 0 and 1 complete together at
                # block (1,1) due to the interleave)
                if (i, p) == (1, 1):
                    emit_outproj(0)
                    emit_outproj(1)
                elif p == 1 and i >= 2:
                    emit_outproj(i)

        while prio or fillers:
            pop_filler()


_NC_CACHE = {}


def _get_nc():
    if "nc" in _NC_CACHE:
        return _NC_CACHE["nc"]
    nc = bacc.Bacc("TRN2", target_bir_lowering=False, debug=False,
                   enable_asserts=False, num_devices=8)
    xt_h = nc.dram_tensor("xt", [E, S], BF16, kind="ExternalInput")
    wq_h = nc.dram_tensor("wq", [E, HDC], BF16, kind="ExternalInput")
    wk_h = nc.dram_tensor("wk", [E, HDC], BF16, kind="ExternalInput")
    wv_h = nc.dram_tensor("wv", [E, HDC], BF16, kind="ExternalInput")
    wo_h = nc.dram_tensor("wo", [HDC, E], BF16, kind="ExternalInput")
    sq_h = nc.dram_tensor("sq", [HDC], F32, kind="ExternalInput")
    so_h = nc.dram_tensor("so", [E], F32, kind="ExternalInput")
    out_h = nc.dram_tensor("out", [S, E], BF16, kind="ExternalOutput")
    with tile.TileContext(nc) as tc:
        build(tc, out_h.ap(), xt_h.ap(), wq_h.ap(), wk_h.ap(), wv_h.ap(),
              wo_h.ap(), sq_h.ap(), so_h.ap())
    nc.compile()
    _NC_CACHE["nc"] = nc
    return nc


def make_in_maps(x, W_qkv, W_out, scale_qkv, scale_out, mask=None):
    import ml_dtypes
    BF = ml_dtypes.bfloat16
    in_maps = []
    sq_full = np.ascontiguousarray(scale_qkv, np.float32).reshape(H, D)
    xts = [np.ascontiguousarray(
        np.asarray(x[b], np.float32).T.astype(BF)) for b in range(B)]
    for b in range(B):
        for g in range(4):
            hs = slice(HC * g, HC * g + HC)
            in_maps.append({
                "xt": xts[b],
                "wq": np.ascontiguousarray(
                    W_qkv[:, 0, hs, :], np.float32).reshape(E, HDC).astype(BF),
                "wk": np.ascontiguousarray(
                    W_qkv[:, 1, hs, :], np.float32).reshape(E, HDC).astype(BF),
                "wv": np.ascontiguousarray(
                    W_qkv[:, 2, hs, :], np.float32).reshape(E, HDC).astype(BF),
                "wo": np.ascontiguousarray(
                    W_out[hs], np.float32).reshape(HDC, E).astype(BF),
                "sq": np.ascontiguousarray(sq_full[hs], np.float32).reshape(HDC),
                "so": np.ascontiguousarray(scale_out, np.float32),
            })
    return in_maps


def kernel(x, W_qkv, W_out, scale_qkv, scale_out, mask=None, _runner_kwargs=None):
    nc = _get_nc()
    in_maps = make_in_maps(x, W_qkv, W_out, scale_qkv, scale_out)
    kw = _runner_kwargs or {}
    res = run_bass_kernel_spmd(nc, in_maps, core_ids=list(range(8)), **kw)
    if _runner_kwargs is not None:
        kernel.last_results = res
    outs = [np.asarray(res.results[i]["out"], np.float32) for i in range(8)]
    full = np.empty((B, S, E), np.float32)
    for b in range(B):
        full[b] = outs[4 * b] + outs[4 * b + 1] + outs[4 * b + 2] + outs[4 * b + 3]
    return full


if __name__ == "__main__":
    rng = np.random.default_rng(0)
    inputs = {
        "x": rng.standard_normal((B, S, E)).astype(np.float32),
        "W_qkv": (rng.standard_normal((E, 3, H, D)).astype(np.float32) * E ** -0.5),
        "W_out": (rng.standard_normal((H, D, E)).astype(np.float32)
                  * (H * D) ** -0.5),
        "scale_qkv": (rng.standard_normal(E).astype(np.float32) * 0.02 + 1.0),
        "scale_out": (rng.standard_normal(E).astype(np.float32) * 0.02 + 1.0),
        "mask": np.tril(np.ones((S, S), bool)),
    }
    out = kernel(**inputs)
    print("kernel ran, out shape", out.shape, out.dtype)
